# revision 23
# baseline (speedup 1.0000x reference)
"""Trainium2 Bass kernel for the GRU/LSTM review-rating model.

Data-parallel over batch: 1024 rows -> 8 NeuronCores x 128 rows.
All activations live "transposed" (feature on partitions, batch on the free
dim) so the recurrent matmuls never need a transpose:
    out[m,b] = sum_k W[k,m] * actT[k,b]   (lhsT = W slice, rhs = actT slice)

Pipeline per core (batch chunk = 128 = partition count):
  phase 0: dma_gather embedding lookup -> xT [128E(pad), T*128] bf16
           (emb col 100 == 1.0 -> layer-1 biases ride the input projection)
           fc1 (upvote head) via K=1 fp32 matmuls
  phase 1: GRU(128) forward + LSTM(128) backward, interleaved, PSUM-fused
  phase 2: GRU(512) + LSTM(512) fused in one loop over t, gates accumulated
           in PSUM [128,2048] (4 m-tiles packed per gate), wide DVE bias-add
           + wide ACT sigmoid/tanh; mean-pool accumulated on DVE
  head:    dense 1024->64->512->128 -> softmax(5) | sigmoid(1) -> out [128,6]
"""

import os
import sys

sys.path.insert(0, "/opt/trn_rl_repo")

import numpy as np
import ml_dtypes

import concourse.bass as bass
import concourse.mybir as mybir
import concourse.tile as tile
from concourse import bacc
from concourse.bass_utils import run_bass_kernel_spmd

BF16 = mybir.dt.bfloat16
F32 = mybir.dt.float32
I16 = mybir.dt.int16

B, T_FULL, V, E = 1024, 100, 9173, 100
H1, H2 = 128, 512
NCORES = 8
BL = B // NCORES  # 128 batch rows per core
AF = mybir.ActivationFunctionType
ALU = mybir.AluOpType
AX = mybir.AxisListType

# LSTM gate reorder: keras [i, f, c, o] -> kernel [i, f, o, c] so that one
# sigmoid covers cols [0 : 3H) and one tanh covers [3H : 4H).
def _lstm_perm(H):
    return np.concatenate(
        [np.arange(0, H), np.arange(H, 2 * H), np.arange(3 * H, 4 * H),
         np.arange(2 * H, 3 * H)]
    )


def _bias_img(bias, nm, bl=BL):
    """[nm*128] bias -> [128, nm*128] image matching packed-m-tile layout:
    img[p, m*128 + b] = bias[m*128 + p]."""
    b = np.asarray(bias, np.float32).reshape(nm, 128).T  # [p, m]
    return np.repeat(b[:, :, None], bl, axis=2).reshape(128, nm * bl)


def build(T):
    """Build the SPMD Bass graph for sequence length T. Returns nc."""
    NT = BL * T
    nc = bacc.Bacc("TRN2", target_bir_lowering=False)

    # ---- DRAM parameters (per-core shards; weights replicated) ----
    dp = nc.declare_dram_parameter
    d_tidx = dp("tidx", [128, NT // 16], I16, isOutput=False)
    d_upv = dp("upvT", [1, BL], F32, isOutput=False)
    d_emb = dp("embp", [V, 128], BF16, isOutput=False)
    d_wg1 = dp("wg1", [128, 3 * H1], BF16, isOutput=False)
    d_wg1r = dp("wg1r", [128, 3 * H1], BF16, isOutput=False)
    d_wl1 = dp("wl1", [128, 4 * H1], BF16, isOutput=False)
    d_wl1r = dp("wl1r", [128, 4 * H1], BF16, isOutput=False)
    d_wg2 = dp("wg2", [128, 2 * 3 * H2], BF16, isOutput=False)
    d_wg2r = dp("wg2r", [128, 4 * 3 * H2], BF16, isOutput=False)
    d_wl2 = dp("wl2", [128, 4 * 4 * H2], BF16, isOutput=False)
    d_wl2r = dp("wl2r", [128, 4 * 4 * H2], BF16, isOutput=False)
    d_bimg = dp("bimg", [128, 4096], F32, isOutput=False)
    d_wfc1 = dp("wfc1", [1, 512], F32, isOutput=False)
    d_bfc1 = dp("bfc1img", [128, 512], F32, isOutput=False)
    d_wd1 = dp("wd1", [128, 512], BF16, isOutput=False)
    d_wd2 = dp("wd2", [64, 512], BF16, isOutput=False)
    d_wd3 = dp("wd3", [128, 512], BF16, isOutput=False)
    d_whead = dp("whead", [128, 6], BF16, isOutput=False)
    d_bhead = dp("bhead", [128, 6], F32, isOutput=False)
    d_biasv = dp("biasv", [128, 8], F32, isOutput=False)
    d_out = dp("out", [128, 6], F32, isOutput=True)

    with tile.TileContext(nc) as tc:
        from contextlib import ExitStack

        with ExitStack() as ctx:
            persist = ctx.enter_context(tc.tile_pool(name="persist", bufs=1))
            ph1 = ExitStack()
            ph2 = ExitStack()

            def load(dram, shape, dtype, name):
                t = persist.tile(shape, dtype, tag=name)
                nc.sync.dma_start(out=t[:, :], in_=dram[:, :])
                return t

            wg1 = load(d_wg1, [128, 3 * H1], BF16, "wg1")
            wg1r = load(d_wg1r, [128, 3 * H1], BF16, "wg1r")
            wl1 = load(d_wl1, [128, 4 * H1], BF16, "wl1")
            wl1r = load(d_wl1r, [128, 4 * H1], BF16, "wl1r")
            wg2 = load(d_wg2, [128, 2 * 3 * H2], BF16, "wg2")
            wg2r = load(d_wg2r, [128, 4 * 3 * H2], BF16, "wg2r")
            wl2 = load(d_wl2, [128, 4 * 4 * H2], BF16, "wl2")
            wl2r = load(d_wl2r, [128, 4 * 4 * H2], BF16, "wl2r")
            bimg = load(d_bimg, [128, 4096], F32, "bimg")
            wfc1 = load(d_wfc1, [1, 512], F32, "wfc1")
            bfc1 = load(d_bfc1, [128, 512], F32, "bfc1")
            wd1 = load(d_wd1, [128, 512], BF16, "wd1")
            wd2 = load(d_wd2, [64, 512], BF16, "wd2")
            wd3 = load(d_wd3, [128, 512], BF16, "wd3")
            whead = load(d_whead, [128, 6], BF16, "whead")
            bhead = load(d_bhead, [128, 6], F32, "bhead")
            biasv = load(d_biasv, [128, 8], F32, "biasv")
            upv = load(d_upv, [1, BL], F32, "upv")

            # persistent state / sequence buffers
            fwd = persist.tile([128, NT], BF16, tag="fwd")   # GRU1 outputs
            bwd = persist.tile([128, NT], BF16, tag="bwd")   # LSTM1 outputs
            x2T = persist.tile([128, 512], BF16, tag="x2T")  # fc1 head
            hLsum = persist.tile([128, 512], F32, tag="hLsum")
            z128b = persist.tile([128, 128], BF16, tag="z128b")
            z512b = persist.tile([128, 512], BF16, tag="z512b")
            z128f = persist.tile([128, 128], F32, tag="z128f")
            z512f = persist.tile([128, 512], F32, tag="z512f")
            outT = persist.tile([128, 6], F32, tag="outT")

            nc.vector.memset(hLsum[:, :], 0.0)
            nc.vector.memset(z128b[:, :], 0.0)
            nc.vector.memset(z512b[:, :], 0.0)
            nc.vector.memset(z128f[:, :], 0.0)
            nc.vector.memset(z512f[:, :], 0.0)

            # ---------------- phase 0: embedding gather + fc1 ----------------
            ph01 = ph1.enter_context(tc.tile_pool(name="ph01", bufs=1))
            ph1ps = ph1.enter_context(
                tc.tile_pool(name="ph1ps", bufs=1, space="PSUM"))
            ph1sb = ph1.enter_context(tc.tile_pool(name="ph1sb", bufs=1))

            tidx = ph01.tile([128, NT // 16], I16, tag="tidx")
            nc.sync.dma_start(out=tidx[:, :], in_=d_tidx[:, :])
            xT = ph01.tile([128, NT], BF16, tag="xT")
            xT3 = xT[:, :].rearrange("p (a n) -> p a n", a=1)

            # gather chunking: keep num_idxs per dma_gather <= 640 (larger
            # chunks were observed to crash the runtime)
            CH = next(d for d in range(1, T + 1)
                      if T % d == 0 and (BL * T) // d <= 640)
            nchunk = NT // CH  # idxs per chunk
            assert nchunk % 128 == 0
            for j in range(CH):
                nc.gpsimd.dma_gather(
                    out_ap=xT3[:, :, j * nchunk:(j + 1) * nchunk],
                    in_ap=d_emb[:, :],
                    idxs_ap=tidx[:, j * (nchunk // 16):(j + 1) * (nchunk // 16)],
                    num_idxs=nchunk,
                    num_idxs_reg=nchunk,
                    elem_size=128,
                    transpose=True,
                )

            # fc1: x2 = relu(upvote @ fc1_w + fc1_b), transposed layout
            psF = ph1ps.tile([128, 512], F32, tag="psF")
            for m in range(4):
                nc.tensor.matmul(
                    psF[:, m * 128:(m + 1) * 128],
                    wfc1[:, m * 128:(m + 1) * 128],
                    upv[:, :], start=True, stop=True)
            tmpF = ph1sb.tile([128, 512], F32, tag="tmpF")
            nc.vector.tensor_tensor(
                out=tmpF[:, :], in0=psF[:, :], in1=bfc1[:, :], op=ALU.add)
            nc.scalar.activation(x2T[:, :], tmpF[:, :], AF.Relu)

            # ---------------- phase 1: GRU1 fwd + LSTM1 bwd ----------------
            ph1sb2 = ph1.enter_context(tc.tile_pool(name="ph1sb2", bufs=2))
            for s in range(T):
                # ---- GRU1, t = s ----
                t = s
                xs = xT[:, t * BL:(t + 1) * BL]
                hprev = z128b[:, :] if t == 0 else fwd[:, (t - 1) * BL:t * BL]
                pg1 = ph1ps.tile([128, 512], F32, tag="pg1", bufs=2)
                for g in range(2):  # z, r: xproj + rec accumulate
                    c0 = g * 128
                    nc.tensor.matmul(pg1[:, c0:c0 + 128], wg1[:, c0:c0 + 128],
                                     xs, start=True, stop=False)
                    nc.tensor.matmul(pg1[:, c0:c0 + 128], wg1r[:, c0:c0 + 128],
                                     hprev, start=False, stop=True)
                nc.tensor.matmul(pg1[:, 256:384], wg1[:, 256:384], xs,
                                 start=True, stop=True)  # hx (incl bias)
                nc.tensor.matmul(pg1[:, 384:512], wg1r[:, 256:384], hprev,
                                 start=True, stop=True)  # hr (no bias)
                zr = ph1sb2.tile([128, 256], F32, tag="zr")
                nc.scalar.activation(zr[:, :], pg1[:, 0:256], AF.Sigmoid)
                t1 = ph1sb2.tile([128, 128], F32, tag="t1")
                # t1 = (hr + br_h) * r
                nc.vector.scalar_tensor_tensor(
                    out=t1[:, :], in0=pg1[:, 384:512], scalar=biasv[:, 0:1],
                    in1=zr[:, 128:256], op0=ALU.add, op1=ALU.mult)
                t2 = ph1sb2.tile([128, 128], F32, tag="t2")
                nc.vector.tensor_tensor(out=t2[:, :], in0=t1[:, :],
                                        in1=pg1[:, 256:384], op=ALU.add)
                hh = ph1sb2.tile([128, 128], F32, tag="hh")
                nc.scalar.activation(hh[:, :], t2[:, :], AF.Tanh)
                dd = ph1sb2.tile([128, 128], F32, tag="dd")
                nc.vector.tensor_tensor(out=dd[:, :], in0=hprev, in1=hh[:, :],
                                        op=ALU.subtract)
                mm = ph1sb2.tile([128, 128], F32, tag="mm")
                nc.vector.tensor_tensor(out=mm[:, :], in0=dd[:, :],
                                        in1=zr[:, 0:128], op=ALU.mult)
                nc.vector.tensor_tensor(out=fwd[:, t * BL:(t + 1) * BL],
                                        in0=mm[:, :], in1=hh[:, :], op=ALU.add)

                # ---- LSTM1 (backward), t = T-1-s ----
                t = T - 1 - s
                xs = xT[:, t * BL:(t + 1) * BL]
                hnext = z128b[:, :] if s == 0 else bwd[:, (t + 1) * BL:(t + 2) * BL]
                pl1 = ph1ps.tile([128, 512], F32, tag="pl1", bufs=2)
                for g in range(4):  # i, f, o, c (reordered)
                    c0 = g * 128
                    nc.tensor.matmul(pl1[:, c0:c0 + 128], wl1[:, c0:c0 + 128],
                                     xs, start=True, stop=False)
                    nc.tensor.matmul(pl1[:, c0:c0 + 128], wl1r[:, c0:c0 + 128],
                                     hnext, start=False, stop=True)
                sif = ph1sb2.tile([128, 384], F32, tag="sif")
                nc.scalar.activation(sif[:, :], pl1[:, 0:384], AF.Sigmoid)
                ctl = ph1sb2.tile([128, 128], F32, tag="ctl")
                nc.scalar.activation(ctl[:, :], pl1[:, 384:512], AF.Tanh)
                cprev1 = z128f[:, :] if s == 0 else c1
                c1n = ph1sb2.tile([128, 128], F32, tag="c1", bufs=3)
                aa = ph1sb2.tile([128, 128], F32, tag="aa")
                nc.vector.tensor_tensor(out=aa[:, :], in0=sif[:, 128:256],
                                        in1=cprev1, op=ALU.mult)
                bb = ph1sb2.tile([128, 128], F32, tag="bb")
                nc.vector.tensor_tensor(out=bb[:, :], in0=sif[:, 0:128],
                                        in1=ctl[:, :], op=ALU.mult)
                nc.vector.tensor_tensor(out=c1n[:, :], in0=aa[:, :],
                                        in1=bb[:, :], op=ALU.add)
                c1 = c1n[:, :]
                tc1 = ph1sb2.tile([128, 128], F32, tag="tc1")
                nc.scalar.activation(tc1[:, :], c1n[:, :], AF.Tanh)
                nc.vector.tensor_tensor(out=bwd[:, t * BL:(t + 1) * BL],
                                        in0=sif[:, 256:384], in1=tc1[:, :],
                                        op=ALU.mult)

            # ---------------- phase 2: GRU2 + LSTM2 fused ----------------
            ph1.close()
            ph2ps = ph2.enter_context(
                tc.tile_pool(name="ph2ps", bufs=1, space="PSUM"))
            ph2sb = ph2.enter_context(tc.tile_pool(name="ph2sb", bufs=1))

            # ping-pong state tiles (step t writes slot t%2)
            h2_pp = [persist.tile([128, 512], BF16, tag=f"h2pp{i}",
                                  name=f"h2pp{i}") for i in range(2)]
            hL_pp = [persist.tile([128, 512], BF16, tag=f"hLpp{i}",
                                  name=f"hLpp{i}") for i in range(2)]
            c2_pp = [persist.tile([128, 512], F32, tag=f"c2pp{i}",
                                  name=f"c2pp{i}") for i in range(2)]

            def g2l2_step(off, h2p, hLp, c2p, h2n, hLn, c2n, first):
                """One fused GRU2+LSTM2 timestep. off: element offset into
                fwd/bwd (int or RuntimeValue); h2p/hLp/c2p prev-state APs;
                h2n/hLn/c2n next-state tiles."""
                fs = fwd[:, bass.ds(off, BL)]
                bs = bwd[:, bass.ds(off, BL)]

                # --- GRU2 matmuls: PG = [z | r | hx | hr], each 512 wide ---
                pg = ph2ps.tile([128, 2048], F32, tag="pg", bufs=1)
                for g in range(2):  # z, r
                    for m in range(4):
                        o = g * 512 + m * 128
                        nc.tensor.matmul(pg[:, o:o + 128],
                                         wg2[:, o:o + 128], fs,
                                         start=True, stop=False)
                        nc.tensor.matmul(pg[:, o:o + 128],
                                         wg2[:, 1536 + o:1536 + o + 128], bs,
                                         start=False, stop=first)
                        if not first:
                            for kt in range(4):
                                nc.tensor.matmul(
                                    pg[:, o:o + 128],
                                    wg2r[:, kt * 1536 + o:kt * 1536 + o + 128],
                                    h2p[:, kt * 128:(kt + 1) * 128],
                                    start=False, stop=(kt == 3))
                for m in range(4):  # hx: xproj only
                    o = 1024 + m * 128
                    nc.tensor.matmul(pg[:, o:o + 128], wg2[:, o:o + 128],
                                     fs, start=True, stop=False)
                    nc.tensor.matmul(pg[:, o:o + 128],
                                     wg2[:, 1536 + o:1536 + o + 128], bs,
                                     start=False, stop=True)
                if not first:
                    for m in range(4):  # hr: rec only
                        o = 1536 + m * 128
                        wc = 1024 + m * 128
                        for kt in range(4):
                            nc.tensor.matmul(
                                pg[:, o:o + 128],
                                wg2r[:, kt * 1536 + wc:kt * 1536 + wc + 128],
                                h2p[:, kt * 128:(kt + 1) * 128],
                                start=(kt == 0), stop=(kt == 3))
                else:
                    nc.vector.memset(pg[:, 1536:2048], 0.0)

                # --- GRU2 gates ---
                zrb = ph2sb.tile([128, 1024], F32, tag="zrb")
                nc.vector.tensor_tensor(out=zrb[:, :], in0=pg[:, 0:1024],
                                        in1=bimg[:, 0:1024], op=ALU.add)
                zr2 = ph2sb.tile([128, 1024], F32, tag="zr2")
                nc.scalar.activation(zr2[:, :], zrb[:, :], AF.Sigmoid)
                hrb = ph2sb.tile([128, 512], F32, tag="hrb")
                nc.vector.tensor_tensor(out=hrb[:, :], in0=pg[:, 1536:2048],
                                        in1=bimg[:, 1536:2048], op=ALU.add)
                t1b = ph2sb.tile([128, 512], F32, tag="t1b")
                nc.vector.tensor_tensor(out=t1b[:, :], in0=hrb[:, :],
                                        in1=zr2[:, 512:1024], op=ALU.mult)
                hxb = ph2sb.tile([128, 512], F32, tag="hxb")
                nc.vector.tensor_tensor(out=hxb[:, :], in0=pg[:, 1024:1536],
                                        in1=bimg[:, 1024:1536], op=ALU.add)
                t2b = ph2sb.tile([128, 512], F32, tag="t2b")
                nc.vector.tensor_tensor(out=t2b[:, :], in0=t1b[:, :],
                                        in1=hxb[:, :], op=ALU.add)
                hh2 = ph2sb.tile([128, 512], F32, tag="hh2")
                nc.scalar.activation(hh2[:, :], t2b[:, :], AF.Tanh)
                dd2 = ph2sb.tile([128, 512], F32, tag="dd2")
                nc.vector.tensor_tensor(out=dd2[:, :], in0=h2p, in1=hh2[:, :],
                                        op=ALU.subtract)
                mm2 = ph2sb.tile([128, 512], F32, tag="mm2")
                nc.vector.tensor_tensor(out=mm2[:, :], in0=dd2[:, :],
                                        in1=zr2[:, 0:512], op=ALU.mult)
                nc.vector.tensor_tensor(out=h2n[:, :], in0=mm2[:, :],
                                        in1=hh2[:, :], op=ALU.add)

                # --- LSTM2 matmuls: PL = [i | f | o | c] ---
                pl = ph2ps.tile([128, 2048], F32, tag="pl", bufs=1)
                for g in range(4):
                    for m in range(4):
                        o = g * 512 + m * 128
                        if not first:
                            for kt in range(4):  # rec first (hLp ready early)
                                nc.tensor.matmul(
                                    pl[:, o:o + 128],
                                    wl2r[:, kt * 2048 + o:kt * 2048 + o + 128],
                                    hLp[:, kt * 128:(kt + 1) * 128],
                                    start=(kt == 0), stop=False)
                        for kt in range(4):  # xproj of fresh h2
                            nc.tensor.matmul(
                                pl[:, o:o + 128],
                                wl2[:, kt * 2048 + o:kt * 2048 + o + 128],
                                h2n[:, kt * 128:(kt + 1) * 128],
                                start=(first and kt == 0), stop=(kt == 3))

                # --- LSTM2 gates ---
                plb = ph2sb.tile([128, 2048], F32, tag="plb")
                nc.vector.tensor_tensor(out=plb[:, :], in0=pl[:, :],
                                        in1=bimg[:, 2048:4096], op=ALU.add)
                sif2 = ph2sb.tile([128, 1536], F32, tag="sif2")
                nc.scalar.activation(sif2[:, :], plb[:, 0:1536], AF.Sigmoid)
                ctl2 = ph2sb.tile([128, 512], F32, tag="ctl2")
                nc.scalar.activation(ctl2[:, :], plb[:, 1536:2048], AF.Tanh)
                bb2 = ph2sb.tile([128, 512], F32, tag="bb2")
                nc.vector.tensor_tensor(out=bb2[:, :], in0=sif2[:, 0:512],
                                        in1=ctl2[:, :], op=ALU.mult)
                if first:
                    nc.vector.tensor_copy(c2n[:, :], bb2[:, :])
                else:
                    aa2 = ph2sb.tile([128, 512], F32, tag="aa2")
                    nc.vector.tensor_tensor(out=aa2[:, :],
                                            in0=sif2[:, 512:1024],
                                            in1=c2p, op=ALU.mult)
                    nc.vector.tensor_tensor(out=c2n[:, :], in0=aa2[:, :],
                                            in1=bb2[:, :], op=ALU.add)
                tc2 = ph2sb.tile([128, 512], F32, tag="tc2")
                nc.scalar.activation(tc2[:, :], c2n[:, :], AF.Tanh)
                nc.vector.tensor_tensor(out=hLn[:, :], in0=sif2[:, 1024:1536],
                                        in1=tc2[:, :], op=ALU.mult)
                nc.vector.tensor_tensor(out=hLsum[:, :], in0=hLsum[:, :],
                                        in1=hLn[:, :], op=ALU.add)

            # peel t = 0, 1; then hardware loop in chunks of U steps
            U = 14
            npeel = 2 if T > 2 else T
            while (T - npeel) % U and npeel < T:
                npeel += 2  # keep parity even
            for t in range(npeel):
                g2l2_step(
                    t * BL,
                    z512b[:, :] if t == 0 else h2_pp[(t + 1) % 2][:, :],
                    z512b[:, :] if t == 0 else hL_pp[(t + 1) % 2][:, :],
                    z512f[:, :] if t == 0 else c2_pp[(t + 1) % 2][:, :],
                    h2_pp[t % 2], hL_pp[t % 2], c2_pp[t % 2],
                    first=(t == 0))
            if npeel < T:
                assert (T - npeel) % U == 0 and npeel % 2 == 0
                with tc.For_i(npeel * BL, T * BL, U * BL,
                              hint_engines=(mybir.EngineType.PE,)) as base:
                    for s in range(U):
                        g2l2_step(
                            base + s * BL,
                            h2_pp[(s + 1) % 2][:, :], hL_pp[(s + 1) % 2][:, :],
                            c2_pp[(s + 1) % 2][:, :],
                            h2_pp[s % 2], hL_pp[s % 2], c2_pp[s % 2],
                            first=False)

            # ---------------- head ----------------
            ph2.close()
            hdps = ctx.enter_context(
                tc.tile_pool(name="hdps", bufs=1, space="PSUM"))
            x1T = persist.tile([128, 512], BF16, tag="x1T")
            nc.vector.tensor_copy(x1T[:, :], hLsum[:, :])

            psD1 = hdps.tile([64, 128], F32, tag="psD1")
            for kt in range(8):
                rhs = (x1T[:, kt * 128:(kt + 1) * 128] if kt < 4
                       else x2T[:, (kt - 4) * 128:(kt - 3) * 128])
                nc.tensor.matmul(psD1[:, :], wd1[:, kt * 64:(kt + 1) * 64],
                                 rhs, start=(kt == 0), stop=(kt == 7))
            hdT = persist.tile([64, 128], BF16, tag="hdT")
            nc.scalar.activation(hdT[:, :], psD1[:, :], AF.Relu,
                                 bias=biasv[0:64, 1:2])

            psD2 = hdps.tile([128, 512], F32, tag="psD2")
            for m in range(4):
                nc.tensor.matmul(psD2[:, m * 128:(m + 1) * 128],
                                 wd2[:, m * 128:(m + 1) * 128], hdT[:, :],
                                 start=True, stop=True)
            h2dT = persist.tile([128, 512], BF16, tag="h2dT")
            for m in range(4):
                nc.scalar.activation(h2dT[:, m * 128:(m + 1) * 128],
                                     psD2[:, m * 128:(m + 1) * 128], AF.Relu,
                                     bias=biasv[:, 3 + m:4 + m])

            psD3 = hdps.tile([128, 128], F32, tag="psD3")
            for kt in range(4):
                nc.tensor.matmul(psD3[:, :], wd3[:, kt * 128:(kt + 1) * 128],
                                 h2dT[:, kt * 128:(kt + 1) * 128],
                                 start=(kt == 0), stop=(kt == 3))
            h3T = persist.tile([128, 128], BF16, tag="h3T")
            nc.scalar.activation(h3T[:, :], psD3[:, :], AF.Relu,
                                 bias=biasv[:, 2:3])

            psH = hdps.tile([128, 6], F32, tag="psH")
            nc.tensor.matmul(psH[:, :], h3T[:, :], whead[:, :],
                             start=True, stop=True)
            hd = persist.tile([128, 6], F32, tag="hd")
            nc.vector.tensor_tensor(out=hd[:, :], in0=psH[:, :],
                                    in1=bhead[:, :], op=ALU.add)

            nc.scalar.activation(outT[:, 5:6], hd[:, 5:6], AF.Sigmoid)
            negmax = persist.tile([128, 1], F32, tag="negmax")
            nc.vector.tensor_reduce(out=negmax[:, :], in_=hd[:, 0:5],
                                    axis=AX.X, op=ALU.max, negate=True)
            ex = persist.tile([128, 5], F32, tag="ex")
            nc.scalar.activation(ex[:, :], hd[:, 0:5], AF.Exp,
                                 bias=negmax[:, :])
            ssum = persist.tile([128, 1], F32, tag="ssum")
            nc.vector.tensor_reduce(out=ssum[:, :], in_=ex[:, :],
                                    axis=AX.X, op=ALU.add)
            rsum = persist.tile([128, 1], F32, tag="rsum")
            nc.vector.reciprocal(rsum[:, :], ssum[:, :])
            nc.vector.tensor_scalar_mul(outT[:, 0:5], ex[:, :], rsum[:, :])

            nc.sync.dma_start(out=d_out[:, :], in_=outT[:, :])

    nc.compile()
    return nc


def _pack_inputs(T, text, upvote, emb, gf_k, gf_rk, gf_b, lb_k, lb_rk, lb_b,
                 g2_k, g2_rk, g2_b, l2_k, l2_rk, l2_b,
                 fc1_w, fc1_b, d1_w, d1_b, d2_w, d2_b, d3_w, d3_b,
                 rat_w, rat_b, rec_w, rec_b):
    bf = ml_dtypes.bfloat16
    f32 = np.float32
    NT = BL * T

    # embedding padded: col 100 == 1.0 (bias row), rest zero
    embp = np.zeros((V, 128), f32)
    embp[:, :E] = emb
    embp[:, E] = 1.0
    embp = embp.astype(bf)

    # GRU1 weights (pad contraction 100->128; bias folded into row 100)
    wg1 = np.zeros((128, 3 * H1), f32)
    wg1[:E] = gf_k
    wg1[E, 0:256] = gf_b[0, 0:256] + gf_b[1, 0:256]   # z, r
    wg1[E, 256:384] = gf_b[0, 256:384]                # h: input bias only
    wg1r = gf_rk.astype(f32)

    # LSTM1 weights reordered [i,f,o,c], bias in row 100
    p1 = _lstm_perm(H1)
    wl1 = np.zeros((128, 4 * H1), f32)
    wl1[:E] = lb_k[:, p1]
    wl1[E] = lb_b[p1]
    wl1r = lb_rk[:, p1].astype(f32)

    # GRU2 weights, k-tile-major
    wg2 = np.concatenate([g2_k[kt * 128:(kt + 1) * 128, :] for kt in range(2)],
                         axis=1)
    wg2r = np.concatenate(
        [g2_rk[kt * 128:(kt + 1) * 128, :] for kt in range(4)], axis=1)
    # LSTM2 reordered + k-tile-major
    p2 = _lstm_perm(H2)
    wl2 = np.concatenate(
        [l2_k[kt * 128:(kt + 1) * 128, p2] for kt in range(4)], axis=1)
    wl2r = np.concatenate(
        [l2_rk[kt * 128:(kt + 1) * 128, p2] for kt in range(4)], axis=1)

    # bias images for phase 2
    bimg = np.concatenate([
        _bias_img(g2_b[0, 0:512] + g2_b[1, 0:512], 4),
        _bias_img(g2_b[0, 512:1024] + g2_b[1, 512:1024], 4),
        _bias_img(g2_b[0, 1024:1536], 4),          # hx: input bias
        _bias_img(g2_b[1, 1024:1536], 4),          # hr: recurrent bias
        _bias_img(l2_b[0:512], 4),                 # i
        _bias_img(l2_b[512:1024], 4),              # f
        _bias_img(l2_b[1536:2048], 4),             # o
        _bias_img(l2_b[1024:1536], 4),             # c
    ], axis=1).astype(f32)

    bfc1img = _bias_img(fc1_b, 4).astype(f32)

    d1_eff = d1_w.astype(np.float64).copy()
    d1_eff[0:512] *= (1.0 / T)                      # mean-pool fold
    wd1 = np.concatenate(
        [d1_eff[kt * 128:(kt + 1) * 128, :] for kt in range(8)],
        axis=1).astype(f32)
    wd2 = d2_w.astype(f32)
    wd3 = np.concatenate(
        [d3_w[kt * 128:(kt + 1) * 128, :] for kt in range(4)], axis=1)
    whead = np.concatenate([rat_w, rec_w], axis=1).astype(f32)
    bhead = np.tile(np.concatenate([rat_b, rec_b])[None, :],
                    (128, 1)).astype(f32)

    biasv = np.zeros((128, 8), f32)
    biasv[:, 0] = gf_b[1, 256:384]                  # GRU1 br_h
    biasv[0:64, 1] = d1_b
    biasv[:, 2] = d3_b
    for m in range(4):
        biasv[:, 3 + m] = d2_b[m * 128:(m + 1) * 128]

    shared = dict(
        embp=embp,
        wg1=wg1.astype(bf), wg1r=wg1r.astype(bf),
        wl1=wl1.astype(bf), wl1r=wl1r.astype(bf),
        wg2=wg2.astype(bf), wg2r=wg2r.astype(bf),
        wl2=wl2.astype(bf), wl2r=wl2r.astype(bf),
        bimg=bimg, wfc1=fc1_w.astype(f32), bfc1img=bfc1img,
        wd1=wd1.astype(bf), wd2=wd2.astype(bf), wd3=wd3.astype(bf),
        whead=whead.astype(bf), bhead=bhead, biasv=biasv,
    )

    in_maps = []
    for c in range(NCORES):
        text_s = np.asarray(text[c * BL:(c + 1) * BL, :T])
        upv_s = np.asarray(upvote[c * BL:(c + 1) * BL, :])
        flat = text_s.T.reshape(-1).astype(np.int16)      # i = t*128 + b
        tidx = np.zeros((128, NT // 16), np.int16)
        tidx[0:16, :] = flat.reshape(NT // 16, 16).T
        m = dict(shared)
        m["tidx"] = tidx
        m["upvT"] = upv_s.T.astype(f32).copy()
        in_maps.append(m)
    return in_maps


_CACHE = {}


def _get_nc(T):
    if T not in _CACHE:
        _CACHE[T] = build(T)
    return _CACHE[T]


def kernel(**inputs):
    T = inputs["text"].shape[1]
    nc = _get_nc(T)
    in_maps = _pack_inputs(T, **inputs)
    res = run_bass_kernel_spmd(nc, in_maps, core_ids=list(range(NCORES)))
    out = np.concatenate([res.results[c]["out"] for c in range(NCORES)], axis=0)
    rating = np.ascontiguousarray(out[:, 0:5], dtype=np.float32)
    recommend = np.ascontiguousarray(out[:, 5:6], dtype=np.float32)
    return rating, recommend


def bench(inputs, iters=10):
    """Steady-state timing of the SPMD NEFF via the bass2jax path with
    device-resident inputs. Returns (best_ns, [per-iter ns], outputs)."""
    import time
    import jax
    from jax.sharding import Mesh, PartitionSpec, NamedSharding
    from jax.experimental.shard_map import shard_map
    from concourse import bass2jax
    import concourse.mybir as mb

    T = inputs["text"].shape[1]
    nc = _get_nc(T)
    in_maps = _pack_inputs(T, **inputs)
    bass2jax.install_neuronx_cc_hook()

    partition_name = (nc.partition_id_tensor.name
                      if nc.partition_id_tensor else None)
    in_names, out_names, out_avals, zero_outs = [], [], [], []
    for alloc in nc.m.functions[0].allocations:
        if not isinstance(alloc, mb.MemoryLocationSet):
            continue
        name = alloc.memorylocations[0].name
        if alloc.kind == "ExternalInput":
            if name != partition_name:
                in_names.append(name)
        elif alloc.kind == "ExternalOutput":
            out_names.append(name)
            shape = tuple(alloc.tensor_shape)
            dtype = mb.dt.np(alloc.dtype)
            out_avals.append(jax.core.ShapedArray(shape, dtype))
            zero_outs.append(np.zeros(shape, dtype))
    n_params = len(in_names)
    n_outs = len(out_avals)
    all_names = in_names + out_names
    if partition_name is not None:
        all_names = all_names + [partition_name]

    def _body(*args):
        operands = list(args)
        if partition_name is not None:
            operands.append(bass2jax.partition_id_tensor())
        outs = bass2jax._bass_exec_p.bind(
            *operands,
            out_avals=tuple(out_avals),
            in_names=tuple(all_names),
            out_names=tuple(out_names),
            lowering_input_output_aliases=(),
            sim_require_finite=True,
            sim_require_nnan=True,
            nc=nc,
        )
        return tuple(outs)

    devices = jax.devices()[:NCORES]
    mesh = Mesh(np.asarray(devices), ("core",))
    in_specs = (PartitionSpec("core"),) * (n_params + n_outs)
    out_specs = (PartitionSpec("core"),) * n_outs
    fn = jax.jit(shard_map(_body, mesh=mesh, in_specs=in_specs,
                           out_specs=out_specs, check_rep=False),
                 keep_unused=True)
    sh = NamedSharding(mesh, PartitionSpec("core"))
    dev_in = [
        jax.device_put(
            np.concatenate([np.asarray(in_maps[c][n]) for c in range(NCORES)],
                           axis=0), sh)
        for n in in_names
    ]
    dev_zero = [
        jax.device_put(np.zeros((NCORES * z.shape[0], *z.shape[1:]), z.dtype),
                       sh) for z in zero_outs
    ]
    out = fn(*dev_in, *dev_zero)  # compile + warm
    jax.block_until_ready(out)
    times = []
    for _ in range(iters):
        t0 = time.perf_counter()
        out = fn(*dev_in, *dev_zero)
        jax.block_until_ready(out)
        times.append((time.perf_counter() - t0) * 1e9)
    outs = np.asarray(out[0]).reshape(NCORES, *out_avals[0].shape)
    full = np.concatenate(list(outs), axis=0)
    return min(times), times, full


# revision 59
# speedup vs baseline: 15.4543x; 15.4543x over previous
"""Trainium2 Bass kernel for the GRU/LSTM review-rating model.

Data-parallel over batch: 1024 rows -> 8 NeuronCores x 128 rows.
All activations live "transposed" (feature on partitions, batch on the free
dim) so the recurrent matmuls never need a transpose:
    out[m,b] = sum_k W[k,m] * actT[k,b]   (lhsT = W slice, rhs = actT slice)

Pipeline per core (batch chunk = 128 = partition count):
  phase 0: dma_gather embedding lookup -> xT [128E(pad), T*128] bf16
           (emb col 100 == 1.0 -> layer-1 biases ride the input projection)
           fc1 (upvote head) via K=1 fp32 matmuls
  phase 1: GRU(128) forward + LSTM(128) backward, interleaved, PSUM-fused
  phase 2: GRU(512) + LSTM(512) fused in one loop over t, gates accumulated
           in PSUM [128,2048] (4 m-tiles packed per gate), wide DVE bias-add
           + wide ACT sigmoid/tanh; mean-pool accumulated on DVE
  head:    dense 1024->64->512->128 -> softmax(5) | sigmoid(1) -> out [128,6]
"""

import os
import sys

sys.path.insert(0, "/opt/trn_rl_repo")

import numpy as np
import ml_dtypes

import concourse.bass as bass
import concourse.mybir as mybir
import concourse.tile as tile
from concourse import bacc
from concourse.bass_utils import run_bass_kernel_spmd

BF16 = mybir.dt.bfloat16
F32 = mybir.dt.float32
I16 = mybir.dt.int16

B, T_FULL, V, E = 1024, 100, 9173, 100
H1, H2 = 128, 512
NCORES = 8
BL = B // NCORES  # 128 batch rows per core
AF = mybir.ActivationFunctionType
ALU = mybir.AluOpType
AX = mybir.AxisListType

# LSTM gate reorder: keras [i, f, c, o] -> kernel [i, f, o, c] so that one
# sigmoid covers cols [0 : 3H) and one tanh covers [3H : 4H).
def _lstm_perm(H):
    return np.concatenate(
        [np.arange(0, H), np.arange(H, 2 * H), np.arange(3 * H, 4 * H),
         np.arange(2 * H, 3 * H)]
    )


def _bias_img(bias, nm, bl=BL):
    """[nm*128] bias -> [128, nm*128] image matching packed-m-tile layout:
    img[p, m*128 + b] = bias[m*128 + p]."""
    b = np.asarray(bias, np.float32).reshape(nm, 128).T  # [p, m]
    return np.repeat(b[:, :, None], bl, axis=2).reshape(128, nm * bl)


def build(T, bench_repeat=1, rep_phase="both", U=14, bias_mode="rank8", sb2=1):
    """Build the SPMD Bass graph for sequence length T. Returns nc.
    bench_repeat>1 wraps phases in an outer repeat loop (timing only).
    rep_phase: which phase the repeat wraps ("both"|"1"|"2").
    U: phase-2 steps per hardware-loop iteration.
    bias_mode: "rank8" = one K=8 N=1024 matmul seeds each psum half with
    the packed gate biases; "k1" = one K=1 matmul per (gate, m-tile);
    "none" = drop phase-2 biases (timing experiments only)."""
    NT = BL * T
    skip_bias = bias_mode == "none"
    fatbias = bias_mode == "rank8"
    sgc = fatbias  # cross-shape accumulation confuses the sim group check
    nc = bacc.Bacc("TRN2", target_bir_lowering=False)

    # ---- DRAM parameters (per-core shards; weights replicated) ----
    dp = nc.declare_dram_parameter
    d_tidx = dp("tidx", [128, NT // 16], I16, isOutput=False)
    d_upv = dp("upvT", [1, BL], F32, isOutput=False)
    d_emb = dp("embp", [V, 128], BF16, isOutput=False)
    d_wg1 = dp("wg1", [128, 3 * H1], BF16, isOutput=False)
    d_wg1r = dp("wg1r", [128, 3 * H1], BF16, isOutput=False)
    d_wl1 = dp("wl1", [128, 4 * H1], BF16, isOutput=False)
    d_wl1r = dp("wl1r", [128, 4 * H1], BF16, isOutput=False)
    d_wg2 = dp("wg2", [128, 2 * 3 * H2], BF16, isOutput=False)
    d_wg2r = dp("wg2r", [128, 4 * 3 * H2], BF16, isOutput=False)
    d_wl2 = dp("wl2", [128, 4 * 4 * H2], BF16, isOutput=False)
    d_wl2r = dp("wl2r", [128, 4 * 4 * H2], BF16, isOutput=False)
    d_g2bl = dp("g2bl", [1, 2048], BF16, isOutput=False)
    d_l2bl = dp("l2bl", [1, 2048], BF16, isOutput=False)
    d_pgb8 = dp("pgb8", [4, 512], BF16, isOutput=False)
    d_plb8 = dp("plb8", [4, 512], BF16, isOutput=False)
    d_ind8 = dp("ind8", [4, 512], BF16, isOutput=False)
    d_g1bl = dp("g1bl", [1, 128], BF16, isOutput=False)
    d_hbias = dp("hbias", [1, 6], BF16, isOutput=False)
    d_wfc1 = dp("wfc1", [1, 512], F32, isOutput=False)
    d_wfc1b = dp("wfc1b", [1, 512], F32, isOutput=False)
    d_wd1 = dp("wd1", [128, 512], BF16, isOutput=False)
    d_wd2 = dp("wd2", [64, 512], BF16, isOutput=False)
    d_wd3 = dp("wd3", [128, 512], BF16, isOutput=False)
    d_whead = dp("whead", [128, 6], BF16, isOutput=False)
    d_biasv = dp("biasv", [128, 8], F32, isOutput=False)
    d_out = dp("out", [128, 6], F32, isOutput=True)

    with tile.TileContext(nc) as tc:
        from contextlib import ExitStack

        with ExitStack() as ctx:
            persist = ctx.enter_context(tc.tile_pool(name="persist", bufs=1))
            ph1 = ExitStack()
            ph2 = ExitStack()

            def load(dram, shape, dtype, name):
                t = persist.tile(shape, dtype, tag=name)
                nc.sync.dma_start(out=t[:, :], in_=dram[:, :])
                return t

            wg1 = load(d_wg1, [128, 3 * H1], BF16, "wg1")
            wg1r = load(d_wg1r, [128, 3 * H1], BF16, "wg1r")
            wl1 = load(d_wl1, [128, 4 * H1], BF16, "wl1")
            wl1r = load(d_wl1r, [128, 4 * H1], BF16, "wl1r")
            wg2 = load(d_wg2, [128, 2 * 3 * H2], BF16, "wg2")
            wg2r = load(d_wg2r, [128, 4 * 3 * H2], BF16, "wg2r")
            wl2 = load(d_wl2, [128, 4 * 4 * H2], BF16, "wl2")
            wl2r = load(d_wl2r, [128, 4 * 4 * H2], BF16, "wl2r")
            g2bl = load(d_g2bl, [1, 2048], BF16, "g2bl")
            l2bl = load(d_l2bl, [1, 2048], BF16, "l2bl")
            pgb8 = load(d_pgb8, [4, 512], BF16, "pgb8")
            plb8 = load(d_plb8, [4, 512], BF16, "plb8")
            ind8 = load(d_ind8, [4, 512], BF16, "ind8")
            g1bl = load(d_g1bl, [1, 128], BF16, "g1bl")
            hbias = load(d_hbias, [1, 6], BF16, "hbias")
            wfc1 = load(d_wfc1, [1, 512], F32, "wfc1")
            wfc1b = load(d_wfc1b, [1, 512], F32, "wfc1b")
            wd1 = load(d_wd1, [128, 512], BF16, "wd1")
            wd2 = load(d_wd2, [64, 512], BF16, "wd2")
            wd3 = load(d_wd3, [128, 512], BF16, "wd3")
            whead = load(d_whead, [128, 6], BF16, "whead")
            biasv = load(d_biasv, [128, 8], F32, "biasv")
            upv = load(d_upv, [1, BL], F32, "upv")

            # persistent state / sequence buffers
            fwd = persist.tile([128, NT], BF16, tag="fwd")   # GRU1 outputs
            bwd = persist.tile([128, NT], BF16, tag="bwd")   # LSTM1 outputs
            x2T = persist.tile([128, 512], BF16, tag="x2T")  # fc1 head
            hLsum = persist.tile([128, 512], F32, tag="hLsum")
            z128b = persist.tile([128, 128], BF16, tag="z128b")
            z512b = persist.tile([128, 512], BF16, tag="z512b")
            z128f = persist.tile([128, 128], F32, tag="z128f")
            z512f = persist.tile([128, 512], F32, tag="z512f")
            outT = persist.tile([128, 6], F32, tag="outT")
            ones1 = persist.tile([1, 128], BF16, tag="ones1")
            ones1f = persist.tile([1, 128], F32, tag="ones1f")

            nc.vector.memset(hLsum[:, :], 0.0)
            nc.vector.memset(z128b[:, :], 0.0)
            nc.vector.memset(z512b[:, :], 0.0)
            nc.vector.memset(z128f[:, :], 0.0)
            nc.vector.memset(z512f[:, :], 0.0)
            nc.vector.memset(ones1[:, :], 1.0)
            nc.vector.memset(ones1f[:, :], 1.0)

            # ---------------- phase 0: embedding gather + fc1 ----------------
            ph01 = ph1.enter_context(tc.tile_pool(name="ph01", bufs=1))
            ph1ps = ph1.enter_context(
                tc.tile_pool(name="ph1ps", bufs=1, space="PSUM"))
            ph1sb = ph1.enter_context(tc.tile_pool(name="ph1sb", bufs=1))

            tidx = ph01.tile([128, NT // 16], I16, tag="tidx")
            nc.sync.dma_start(out=tidx[:, :], in_=d_tidx[:, :])
            xT = ph01.tile([128, NT], BF16, tag="xT")
            xT3 = xT[:, :].rearrange("p (a n) -> p a n", a=1)

            # gather chunking: keep num_idxs per dma_gather <= 640 (larger
            # chunks were observed to crash the runtime)
            CH = next(d for d in range(1, T + 1)
                      if T % d == 0 and (BL * T) // d <= 640)
            nchunk = NT // CH  # idxs per chunk
            assert nchunk % 128 == 0
            for j in range(CH):
                nc.gpsimd.dma_gather(
                    out_ap=xT3[:, :, j * nchunk:(j + 1) * nchunk],
                    in_ap=d_emb[:, :],
                    idxs_ap=tidx[:, j * (nchunk // 16):(j + 1) * (nchunk // 16)],
                    num_idxs=nchunk,
                    num_idxs_reg=nchunk,
                    elem_size=128,
                    transpose=True,
                )

            # fc1: x2 = relu(upvote @ fc1_w + fc1_b), transposed layout
            psF = ph1ps.tile([128, 512], F32, tag="psF")
            for m in range(4):
                nc.tensor.matmul(
                    psF[:, m * 128:(m + 1) * 128],
                    wfc1[:, m * 128:(m + 1) * 128],
                    upv[:, :], start=True, stop=False)
                nc.tensor.matmul(
                    psF[:, m * 128:(m + 1) * 128],
                    wfc1b[:, m * 128:(m + 1) * 128],
                    ones1f[:, :], start=False, stop=True)
            nc.scalar.activation(x2T[:, :], psF[:, :], AF.Relu)

            # ---------------- phase 1: GRU1 fwd + LSTM1 bwd ----------------
            ph1sb2 = ph1.enter_context(tc.tile_pool(name="ph1sb2", bufs=2))
            rep1 = ExitStack()
            if bench_repeat > 1 and rep_phase in ("both", "1"):
                rep1.enter_context(tc.For_i(0, bench_repeat, 1))
            for s in range(T):
                # ---- GRU1, t = s ----
                t = s
                xs = xT[:, t * BL:(t + 1) * BL]
                hprev = z128b[:, :] if t == 0 else fwd[:, (t - 1) * BL:t * BL]
                pg1 = ph1ps.tile([128, 512], F32, tag="pg1", bufs=2)
                for g in range(2):  # z, r: xproj + rec accumulate
                    c0 = g * 128
                    nc.tensor.matmul(pg1[:, c0:c0 + 128], wg1[:, c0:c0 + 128],
                                     xs, start=True, stop=False)
                    nc.tensor.matmul(pg1[:, c0:c0 + 128], wg1r[:, c0:c0 + 128],
                                     hprev, start=False, stop=True)
                nc.tensor.matmul(pg1[:, 256:384], wg1[:, 256:384], xs,
                                 start=True, stop=True)  # hx (incl bias)
                nc.tensor.matmul(pg1[:, 384:512], g1bl[:, :], ones1[:, :],
                                 start=True, stop=(t == 0))  # hr bias br_h
                if t > 0:
                    nc.tensor.matmul(pg1[:, 384:512], wg1r[:, 256:384], hprev,
                                     start=False, stop=True)  # hr
                zr = ph1sb2.tile([128, 256], F32, tag="zr")
                nc.scalar.activation(zr[:, :], pg1[:, 0:256], AF.Sigmoid)
                t1 = ph1sb2.tile([128, 128], F32, tag="t1")
                # t1 = (hr + br_h) * r
                nc.vector.tensor_tensor(
                    out=t1[:, :], in0=pg1[:, 384:512],
                    in1=zr[:, 128:256], op=ALU.mult)
                t2 = ph1sb2.tile([128, 128], F32, tag="t2")
                nc.vector.tensor_tensor(out=t2[:, :], in0=t1[:, :],
                                        in1=pg1[:, 256:384], op=ALU.add)
                hh = ph1sb2.tile([128, 128], F32, tag="hh")
                nc.scalar.activation(hh[:, :], t2[:, :], AF.Tanh)
                dd = ph1sb2.tile([128, 128], F32, tag="dd")
                nc.vector.tensor_tensor(out=dd[:, :], in0=hprev, in1=hh[:, :],
                                        op=ALU.subtract)
                mm = ph1sb2.tile([128, 128], F32, tag="mm")
                nc.vector.tensor_tensor(out=mm[:, :], in0=dd[:, :],
                                        in1=zr[:, 0:128], op=ALU.mult)
                nc.vector.tensor_tensor(out=fwd[:, t * BL:(t + 1) * BL],
                                        in0=mm[:, :], in1=hh[:, :], op=ALU.add)

                # ---- LSTM1 (backward), t = T-1-s ----
                t = T - 1 - s
                xs = xT[:, t * BL:(t + 1) * BL]
                hnext = z128b[:, :] if s == 0 else bwd[:, (t + 1) * BL:(t + 2) * BL]
                pl1 = ph1ps.tile([128, 512], F32, tag="pl1", bufs=2)
                for g in range(4):  # i, f, o, c (reordered)
                    c0 = g * 128
                    nc.tensor.matmul(pl1[:, c0:c0 + 128], wl1[:, c0:c0 + 128],
                                     xs, start=True, stop=False)
                    nc.tensor.matmul(pl1[:, c0:c0 + 128], wl1r[:, c0:c0 + 128],
                                     hnext, start=False, stop=True)
                sif = ph1sb2.tile([128, 384], F32, tag="sif")
                nc.scalar.activation(sif[:, :], pl1[:, 0:384], AF.Sigmoid)
                ctl = ph1sb2.tile([128, 128], F32, tag="ctl")
                nc.scalar.activation(ctl[:, :], pl1[:, 384:512], AF.Tanh)
                cprev1 = z128f[:, :] if s == 0 else c1
                c1n = ph1sb2.tile([128, 128], F32, tag="c1", bufs=3)
                aa = ph1sb2.tile([128, 128], F32, tag="aa")
                nc.vector.tensor_tensor(out=aa[:, :], in0=sif[:, 128:256],
                                        in1=cprev1, op=ALU.mult)
                bb = ph1sb2.tile([128, 128], F32, tag="bb")
                nc.vector.tensor_tensor(out=bb[:, :], in0=sif[:, 0:128],
                                        in1=ctl[:, :], op=ALU.mult)
                nc.vector.tensor_tensor(out=c1n[:, :], in0=aa[:, :],
                                        in1=bb[:, :], op=ALU.add)
                c1 = c1n[:, :]
                tc1 = ph1sb2.tile([128, 128], F32, tag="tc1")
                nc.scalar.activation(tc1[:, :], c1n[:, :], AF.Tanh)
                nc.vector.tensor_tensor(out=bwd[:, t * BL:(t + 1) * BL],
                                        in0=sif[:, 256:384], in1=tc1[:, :],
                                        op=ALU.mult)

            # ---------------- phase 2: GRU2 + LSTM2 fused ----------------
            rep1.close()
            ph1.close()
            ph2ps = ph2.enter_context(
                tc.tile_pool(name="ph2ps", bufs=1, space="PSUM"))
            ph2sb = ph2.enter_context(tc.tile_pool(name="ph2sb", bufs=sb2))

            # ping-pong state tiles (step t writes slot t%2)
            h2_pp = [persist.tile([128, 512], BF16, tag=f"h2pp{i}",
                                  name=f"h2pp{i}") for i in range(2)]
            hL_pp = [persist.tile([128, 512], BF16, tag=f"hLpp{i}",
                                  name=f"hLpp{i}") for i in range(2)]
            c2_pp = [persist.tile([128, 512], F32, tag=f"c2pp{i}",
                                  name=f"c2pp{i}") for i in range(2)]

            def mm_group(out_ap, mms, seeded=False):
                """Emit an accumulation group with start/stop on first/last.
                seeded: the psum region was already written by a wide bias
                matmul, so never emit start=True here."""
                for i, (lhsT, rhs) in enumerate(mms):
                    nc.tensor.matmul(out_ap, lhsT, rhs,
                                     start=(i == 0 and not seeded),
                                     stop=(i == len(mms) - 1),
                                     skip_group_check=sgc)

            def gru2_step(off, h2p, h2n, first):
                """GRU2 timestep. off: element offset into fwd/bwd (int or
                RuntimeValue); h2p prev-state AP; h2n next-state tile."""
                fs = fwd[:, bass.ds(off, BL)]
                bs = bwd[:, bass.ds(off, BL)]

                # --- GRU2 matmuls: PG = [z | r | hx | hr], each 512 wide ---
                # biases ride the PSUM accumulation: either one K=8 N=1024
                # seeding matmul per psum half, or K=1 matmuls per m-tile
                pg = ph2ps.tile([128, 2048], F32, tag="pg", bufs=1)
                if fatbias:
                    for h in range(4):
                        nc.tensor.matmul(pg[:, h * 512:(h + 1) * 512],
                                         pgb8[:, h * 128:(h + 1) * 128],
                                         ind8[:, :], start=True, stop=False,
                                         skip_group_check=True)
                for g in range(2):  # z, r
                    for m in range(4):
                        o = g * 512 + m * 128
                        mms = []
                        if not skip_bias and not fatbias:
                            mms.append((g2bl[:, o:o + 128], ones1[:, :]))
                        mms.append((wg2[:, o:o + 128], fs))
                        mms.append((wg2[:, 1536 + o:1536 + o + 128], bs))
                        if not first:
                            for kt in range(4):
                                mms.append((
                                    wg2r[:, kt * 1536 + o:kt * 1536 + o + 128],
                                    h2p[:, kt * 128:(kt + 1) * 128]))
                        mm_group(pg[:, o:o + 128], mms, seeded=fatbias)
                for m in range(4):  # hx: xproj only
                    o = 1024 + m * 128
                    mms = []
                    if not skip_bias and not fatbias:
                        mms.append((g2bl[:, o:o + 128], ones1[:, :]))
                    mms.append((wg2[:, o:o + 128], fs))
                    mms.append((wg2[:, 1536 + o:1536 + o + 128], bs))
                    mm_group(pg[:, o:o + 128], mms, seeded=fatbias)
                for m in range(4):  # hr: rec only (+ bias br_h)
                    o = 1536 + m * 128
                    wc = 1024 + m * 128
                    mms = [] if fatbias else [(g2bl[:, o:o + 128], ones1[:, :])]
                    if not first:
                        for kt in range(4):
                            mms.append((
                                wg2r[:, kt * 1536 + wc:kt * 1536 + wc + 128],
                                h2p[:, kt * 128:(kt + 1) * 128]))
                    if mms:
                        mm_group(pg[:, o:o + 128], mms, seeded=fatbias)

                # --- GRU2 gates ---
                zr2 = ph2sb.tile([128, 1024], F32, tag="zr2")
                nc.scalar.activation(zr2[:, :], pg[:, 0:1024], AF.Sigmoid)
                t1b = ph2sb.tile([128, 512], F32, tag="t1b")
                nc.vector.tensor_tensor(out=t1b[:, :], in0=pg[:, 1536:2048],
                                        in1=zr2[:, 512:1024], op=ALU.mult)
                t2b = ph2sb.tile([128, 512], F32, tag="t2b")
                nc.vector.tensor_tensor(out=t2b[:, :], in0=t1b[:, :],
                                        in1=pg[:, 1024:1536], op=ALU.add)
                hh2 = ph2sb.tile([128, 512], F32, tag="hh2")
                nc.scalar.activation(hh2[:, :], t2b[:, :], AF.Tanh)
                dd2 = ph2sb.tile([128, 512], F32, tag="dd2")
                nc.vector.tensor_tensor(out=dd2[:, :], in0=h2p, in1=hh2[:, :],
                                        op=ALU.subtract)
                mm2 = ph2sb.tile([128, 512], F32, tag="mm2")
                nc.vector.tensor_tensor(out=mm2[:, :], in0=dd2[:, :],
                                        in1=zr2[:, 0:512], op=ALU.mult)
                nc.vector.tensor_tensor(out=h2n[:, :], in0=mm2[:, :],
                                        in1=hh2[:, :], op=ALU.add)

            def lstm2_step(h2in, hLp, c2p, hLn, c2n, first):
                """LSTM2 timestep (emitted one step behind GRU2 so the PE
                always has a ready batch of matmuls while GRU2's gate chain
                completes). All operands are static tiles."""
                # --- LSTM2 matmuls: PL = [i | f | o | c] ---
                pl = ph2ps.tile([128, 2048], F32, tag="pl", bufs=1)
                if fatbias:
                    for h in range(4):
                        nc.tensor.matmul(pl[:, h * 512:(h + 1) * 512],
                                         plb8[:, h * 128:(h + 1) * 128],
                                         ind8[:, :], start=True, stop=False,
                                         skip_group_check=True)
                for g in range(4):
                    for m in range(4):
                        o = g * 512 + m * 128
                        mms = []
                        if not skip_bias and not fatbias:
                            mms.append((l2bl[:, o:o + 128], ones1[:, :]))
                        if not first:
                            for kt in range(4):  # rec first (hLp ready early)
                                mms.append((
                                    wl2r[:, kt * 2048 + o:kt * 2048 + o + 128],
                                    hLp[:, kt * 128:(kt + 1) * 128]))
                        for kt in range(4):  # xproj of this step's h2
                            mms.append((
                                wl2[:, kt * 2048 + o:kt * 2048 + o + 128],
                                h2in[:, kt * 128:(kt + 1) * 128]))
                        mm_group(pl[:, o:o + 128], mms, seeded=fatbias)

                # --- LSTM2 gates ---
                sif2 = ph2sb.tile([128, 1536], F32, tag="sif2")
                nc.scalar.activation(sif2[:, :], pl[:, 0:1536], AF.Sigmoid)
                ctl2 = ph2sb.tile([128, 512], F32, tag="ctl2")
                nc.scalar.activation(ctl2[:, :], pl[:, 1536:2048], AF.Tanh)
                bb2 = ph2sb.tile([128, 512], F32, tag="bb2")
                nc.gpsimd.tensor_tensor(out=bb2[:, :], in0=sif2[:, 0:512],
                                        in1=ctl2[:, :], op=ALU.mult)
                if first:
                    nc.gpsimd.tensor_copy(c2n[:, :], bb2[:, :])
                else:
                    aa2 = ph2sb.tile([128, 512], F32, tag="aa2")
                    nc.gpsimd.tensor_tensor(out=aa2[:, :],
                                            in0=sif2[:, 512:1024],
                                            in1=c2p, op=ALU.mult)
                    nc.gpsimd.tensor_tensor(out=c2n[:, :], in0=aa2[:, :],
                                            in1=bb2[:, :], op=ALU.add)
                tc2 = ph2sb.tile([128, 512], F32, tag="tc2")
                nc.scalar.activation(tc2[:, :], c2n[:, :], AF.Tanh)
                nc.vector.tensor_tensor(out=hLn[:, :], in0=sif2[:, 1024:1536],
                                        in1=tc2[:, :], op=ALU.mult)
                nc.gpsimd.tensor_tensor(out=hLsum[:, :], in0=hLsum[:, :],
                                        in1=hLn[:, :], op=ALU.add)

            # peel t = 0..npeel-1; hardware loop in chunks of U steps.
            # LSTM2 runs one step behind GRU2 in program order so the PE
            # stream never waits on the fresh GRU2 state.
            rep2 = ExitStack()
            if bench_repeat > 1 and rep_phase in ("both", "2"):
                rep2.enter_context(tc.For_i(0, bench_repeat, 1))
            npeel = 2 if T > 2 else T
            while (T - npeel) % U and npeel < T:
                npeel += 2  # keep parity even
            for t in range(npeel):
                gru2_step(
                    t * BL,
                    z512b[:, :] if t == 0 else h2_pp[(t + 1) % 2][:, :],
                    h2_pp[t % 2], first=(t == 0))
                if t > 0:
                    tl = t - 1
                    lstm2_step(
                        h2_pp[tl % 2][:, :],
                        z512b[:, :] if tl == 0 else hL_pp[(tl + 1) % 2][:, :],
                        z512f[:, :] if tl == 0 else c2_pp[(tl + 1) % 2][:, :],
                        hL_pp[tl % 2], c2_pp[tl % 2], first=(tl == 0))
            if npeel < T:
                assert (T - npeel) % U == 0 and npeel % 2 == 0
                with tc.For_i(npeel * BL, T * BL, U * BL,
                              hint_engines=(mybir.EngineType.PE,)) as base:
                    for s in range(U):
                        gru2_step(base + s * BL, h2_pp[(s + 1) % 2][:, :],
                                  h2_pp[s % 2], first=False)
                        # lstm2 for t-1 = base/BL + s - 1, parity (s+1)%2
                        lstm2_step(h2_pp[(s + 1) % 2][:, :],
                                   hL_pp[s % 2][:, :], c2_pp[s % 2][:, :],
                                   hL_pp[(s + 1) % 2], c2_pp[(s + 1) % 2],
                                   first=False)
            # tail: lstm2 for t = T-1
            tl = T - 1
            lstm2_step(h2_pp[tl % 2][:, :], hL_pp[(tl + 1) % 2][:, :],
                       c2_pp[(tl + 1) % 2][:, :],
                       hL_pp[tl % 2], c2_pp[tl % 2], first=(tl == 0))
            rep2.close()

            # ---------------- head ----------------
            ph2.close()
            hdps = ctx.enter_context(
                tc.tile_pool(name="hdps", bufs=1, space="PSUM"))
            x1T = persist.tile([128, 512], BF16, tag="x1T")
            nc.vector.tensor_copy(x1T[:, :], hLsum[:, :])

            psD1 = hdps.tile([64, 128], F32, tag="psD1")
            for kt in range(8):
                rhs = (x1T[:, kt * 128:(kt + 1) * 128] if kt < 4
                       else x2T[:, (kt - 4) * 128:(kt - 3) * 128])
                nc.tensor.matmul(psD1[:, :], wd1[:, kt * 64:(kt + 1) * 64],
                                 rhs, start=(kt == 0), stop=(kt == 7))
            hdT = persist.tile([64, 128], BF16, tag="hdT")
            nc.scalar.activation(hdT[:, :], psD1[:, :], AF.Relu,
                                 bias=biasv[0:64, 1:2])

            psD2 = hdps.tile([128, 512], F32, tag="psD2")
            for m in range(4):
                nc.tensor.matmul(psD2[:, m * 128:(m + 1) * 128],
                                 wd2[:, m * 128:(m + 1) * 128], hdT[:, :],
                                 start=True, stop=True)
            h2dT = persist.tile([128, 512], BF16, tag="h2dT")
            for m in range(4):
                nc.scalar.activation(h2dT[:, m * 128:(m + 1) * 128],
                                     psD2[:, m * 128:(m + 1) * 128], AF.Relu,
                                     bias=biasv[:, 3 + m:4 + m])

            psD3 = hdps.tile([128, 128], F32, tag="psD3")
            for kt in range(4):
                nc.tensor.matmul(psD3[:, :], wd3[:, kt * 128:(kt + 1) * 128],
                                 h2dT[:, kt * 128:(kt + 1) * 128],
                                 start=(kt == 0), stop=(kt == 3))
            h3T = persist.tile([128, 128], BF16, tag="h3T")
            nc.scalar.activation(h3T[:, :], psD3[:, :], AF.Relu,
                                 bias=biasv[:, 2:3])

            psH = hdps.tile([128, 6], F32, tag="psH")
            nc.tensor.matmul(psH[:, :], h3T[:, :], whead[:, :],
                             start=True, stop=False)
            nc.tensor.matmul(psH[:, :], ones1[:, :], hbias[:, :],
                             start=False, stop=True)

            nc.scalar.activation(outT[:, 5:6], psH[:, 5:6], AF.Sigmoid)
            negmax = persist.tile([128, 1], F32, tag="negmax")
            nc.vector.tensor_reduce(out=negmax[:, :], in_=psH[:, 0:5],
                                    axis=AX.X, op=ALU.max, negate=True)
            ex = persist.tile([128, 5], F32, tag="ex")
            nc.scalar.activation(ex[:, :], psH[:, 0:5], AF.Exp,
                                 bias=negmax[:, :])
            ssum = persist.tile([128, 1], F32, tag="ssum")
            nc.vector.tensor_reduce(out=ssum[:, :], in_=ex[:, :],
                                    axis=AX.X, op=ALU.add)
            rsum = persist.tile([128, 1], F32, tag="rsum")
            nc.vector.reciprocal(rsum[:, :], ssum[:, :])
            nc.vector.tensor_scalar_mul(outT[:, 0:5], ex[:, :], rsum[:, :])

            nc.sync.dma_start(out=d_out[:, :], in_=outT[:, :])

    nc.compile()
    return nc


def _pack_inputs(T, text, upvote, emb, gf_k, gf_rk, gf_b, lb_k, lb_rk, lb_b,
                 g2_k, g2_rk, g2_b, l2_k, l2_rk, l2_b,
                 fc1_w, fc1_b, d1_w, d1_b, d2_w, d2_b, d3_w, d3_b,
                 rat_w, rat_b, rec_w, rec_b):
    bf = ml_dtypes.bfloat16
    f32 = np.float32
    NT = BL * T

    # embedding padded: col 100 == 1.0 (bias row), rest zero
    embp = np.zeros((V, 128), f32)
    embp[:, :E] = emb
    embp[:, E] = 1.0
    embp = embp.astype(bf)

    # GRU1 weights (pad contraction 100->128; bias folded into row 100)
    wg1 = np.zeros((128, 3 * H1), f32)
    wg1[:E] = gf_k
    wg1[E, 0:256] = gf_b[0, 0:256] + gf_b[1, 0:256]   # z, r
    wg1[E, 256:384] = gf_b[0, 256:384]                # h: input bias only
    wg1r = gf_rk.astype(f32)

    # LSTM1 weights reordered [i,f,o,c], bias in row 100
    p1 = _lstm_perm(H1)
    wl1 = np.zeros((128, 4 * H1), f32)
    wl1[:E] = lb_k[:, p1]
    wl1[E] = lb_b[p1]
    wl1r = lb_rk[:, p1].astype(f32)

    # GRU2 weights, k-tile-major
    wg2 = np.concatenate([g2_k[kt * 128:(kt + 1) * 128, :] for kt in range(2)],
                         axis=1)
    wg2r = np.concatenate(
        [g2_rk[kt * 128:(kt + 1) * 128, :] for kt in range(4)], axis=1)
    # LSTM2 reordered + k-tile-major
    p2 = _lstm_perm(H2)
    wl2 = np.concatenate(
        [l2_k[kt * 128:(kt + 1) * 128, p2] for kt in range(4)], axis=1)
    wl2r = np.concatenate(
        [l2_rk[kt * 128:(kt + 1) * 128, p2] for kt in range(4)], axis=1)

    # phase-2 bias rows: applied via K=1 matmuls into the PSUM groups
    g2bl = np.concatenate([
        g2_b[0, 0:512] + g2_b[1, 0:512],           # z
        g2_b[0, 512:1024] + g2_b[1, 512:1024],     # r
        g2_b[0, 1024:1536],                        # hx: input bias
        g2_b[1, 1024:1536],                        # hr: recurrent bias
    ])[None, :]
    l2bl = np.concatenate([
        l2_b[0:512], l2_b[512:1024],               # i, f
        l2_b[1536:2048], l2_b[1024:1536],          # o, c (reordered)
    ])[None, :]
    g1bl = gf_b[1, 256:384][None, :]               # GRU1 br_h
    hbias = np.concatenate([rat_b, rec_b])[None, :]

    # rank-4 bias seeds: pgb8[k, b*128+p] = bias[b*512 + k*128 + p]
    def _seed(flat):
        return np.asarray(flat).reshape(4, 4, 128).transpose(1, 0, 2).reshape(4, 512)
    pgb8 = _seed(g2bl[0])
    plb8 = _seed(l2bl[0])
    ind8 = np.zeros((4, 512), np.float32)
    for k in range(4):
        ind8[k, k * 128:(k + 1) * 128] = 1.0

    d1_eff = d1_w.astype(np.float64).copy()
    d1_eff[0:512] *= (1.0 / T)                      # mean-pool fold
    wd1 = np.concatenate(
        [d1_eff[kt * 128:(kt + 1) * 128, :] for kt in range(8)],
        axis=1).astype(f32)
    wd2 = d2_w.astype(f32)
    wd3 = np.concatenate(
        [d3_w[kt * 128:(kt + 1) * 128, :] for kt in range(4)], axis=1)
    whead = np.concatenate([rat_w, rec_w], axis=1).astype(f32)

    biasv = np.zeros((128, 8), f32)
    biasv[0:64, 1] = d1_b
    biasv[:, 2] = d3_b
    for m in range(4):
        biasv[:, 3 + m] = d2_b[m * 128:(m + 1) * 128]

    shared = dict(
        embp=embp,
        wg1=wg1.astype(bf), wg1r=wg1r.astype(bf),
        wl1=wl1.astype(bf), wl1r=wl1r.astype(bf),
        wg2=wg2.astype(bf), wg2r=wg2r.astype(bf),
        wl2=wl2.astype(bf), wl2r=wl2r.astype(bf),
        g2bl=g2bl.astype(bf), l2bl=l2bl.astype(bf), g1bl=g1bl.astype(bf),
        hbias=hbias.astype(bf), pgb8=pgb8.astype(bf), plb8=plb8.astype(bf),
        ind8=ind8.astype(bf),
        wfc1=fc1_w.astype(f32), wfc1b=fc1_b[None, :].astype(f32),
        wd1=wd1.astype(bf), wd2=wd2.astype(bf), wd3=wd3.astype(bf),
        whead=whead.astype(bf), biasv=biasv,
    )

    in_maps = []
    for c in range(NCORES):
        text_s = np.asarray(text[c * BL:(c + 1) * BL, :T])
        upv_s = np.asarray(upvote[c * BL:(c + 1) * BL, :])
        flat = text_s.T.reshape(-1).astype(np.int16)      # i = t*128 + b
        tidx = np.zeros((128, NT // 16), np.int16)
        tidx[0:16, :] = flat.reshape(NT // 16, 16).T
        m = dict(shared)
        m["tidx"] = tidx
        m["upvT"] = upv_s.T.astype(f32).copy()
        in_maps.append(m)
    return in_maps


_CACHE = {}


def _get_nc(T):
    if T not in _CACHE:
        _CACHE[T] = build(T)
    return _CACHE[T]


def kernel(**inputs):
    T = inputs["text"].shape[1]
    nc = _get_nc(T)
    in_maps = _pack_inputs(T, **inputs)
    res = run_bass_kernel_spmd(nc, in_maps, core_ids=list(range(NCORES)))
    out = np.concatenate([res.results[c]["out"] for c in range(NCORES)], axis=0)
    rating = np.ascontiguousarray(out[:, 0:5], dtype=np.float32)
    recommend = np.ascontiguousarray(out[:, 5:6], dtype=np.float32)
    return rating, recommend


def bench(inputs, iters=10):
    """Steady-state timing of the SPMD NEFF via the bass2jax path with
    device-resident inputs. Returns (best_ns, [per-iter ns], outputs)."""
    T = inputs["text"].shape[1]
    nc = _get_nc(T)
    in_maps = _pack_inputs(T, **inputs)
    return bench_nc(nc, in_maps, iters)


def bench_nc(nc, in_maps, iters=10):
    import time
    import jax
    from jax.sharding import Mesh, PartitionSpec, NamedSharding
    from jax.experimental.shard_map import shard_map
    from concourse import bass2jax
    import concourse.mybir as mb

    bass2jax.install_neuronx_cc_hook()

    partition_name = (nc.partition_id_tensor.name
                      if nc.partition_id_tensor else None)
    in_names, out_names, out_avals, zero_outs = [], [], [], []
    for alloc in nc.m.functions[0].allocations:
        if not isinstance(alloc, mb.MemoryLocationSet):
            continue
        name = alloc.memorylocations[0].name
        if alloc.kind == "ExternalInput":
            if name != partition_name:
                in_names.append(name)
        elif alloc.kind == "ExternalOutput":
            out_names.append(name)
            shape = tuple(alloc.tensor_shape)
            dtype = mb.dt.np(alloc.dtype)
            out_avals.append(jax.core.ShapedArray(shape, dtype))
            zero_outs.append(np.zeros(shape, dtype))
    n_params = len(in_names)
    n_outs = len(out_avals)
    all_names = in_names + out_names
    if partition_name is not None:
        all_names = all_names + [partition_name]

    def _body(*args):
        operands = list(args)
        if partition_name is not None:
            operands.append(bass2jax.partition_id_tensor())
        outs = bass2jax._bass_exec_p.bind(
            *operands,
            out_avals=tuple(out_avals),
            in_names=tuple(all_names),
            out_names=tuple(out_names),
            lowering_input_output_aliases=(),
            sim_require_finite=True,
            sim_require_nnan=True,
            nc=nc,
        )
        return tuple(outs)

    devices = jax.devices()[:NCORES]
    mesh = Mesh(np.asarray(devices), ("core",))
    in_specs = (PartitionSpec("core"),) * (n_params + n_outs)
    out_specs = (PartitionSpec("core"),) * n_outs
    fn = jax.jit(shard_map(_body, mesh=mesh, in_specs=in_specs,
                           out_specs=out_specs, check_rep=False),
                 keep_unused=True)
    sh = NamedSharding(mesh, PartitionSpec("core"))
    dev_in = [
        jax.device_put(
            np.concatenate([np.asarray(in_maps[c][n]) for c in range(NCORES)],
                           axis=0), sh)
        for n in in_names
    ]
    dev_zero = [
        jax.device_put(np.zeros((NCORES * z.shape[0], *z.shape[1:]), z.dtype),
                       sh) for z in zero_outs
    ]
    out = fn(*dev_in, *dev_zero)  # compile + warm
    jax.block_until_ready(out)
    times = []
    for _ in range(iters):
        t0 = time.perf_counter()
        out = fn(*dev_in, *dev_zero)
        jax.block_until_ready(out)
        times.append((time.perf_counter() - t0) * 1e9)
    outs = np.asarray(out[0]).reshape(NCORES, *out_avals[0].shape)
    full = np.concatenate(list(outs), axis=0)
    return min(times), times, full
